# revision 1
# baseline (speedup 1.0000x reference)
"""Trainium2 Bass kernel for nn_MoEConnectionProcessor.

Self-contained: stages/shards the full inputs on host (numpy), runs an SPMD
Bass/Tile kernel on 8 NeuronCores, gathers the full output.

Reference math (per cell, K=26 neighbors, D=32):
  masks by tier (0=local,1=functional,2=distant); masked neighbor means;
  local expert  = tanh([cs, loc_mean] @ W_local + b_local)
  func expert   = (1-z)*cs + z*tanh(agg),  z = sigmoid([cs, agg] @ W_upd + b_upd)
                  agg = masked_mean_k tanh(nb @ W_msg + b_msg)
  dist expert   = 3-step Euler: x += (1/3) tanh([x, agg_d] @ W_cnf + b_cnf)
  gates         = softmax([cs, mean_nb] @ W_g1 + b_g1 -> relu -> @ W_g2 + b_g2)
  out           = sum_t gate_t * expert_t

Device layout strategy per 128-cell tile (cells on SBUF partitions):
  - neighbor data staged natural [cells, (k d)] bf16; DVE StreamTranspose
    gives the d-on-partition operand for the PE matmul with a 4x block
    diagonal W_msg (contraction=32 features x 4 cell subgroups).
  - masked k-sums: DVE broadcast-AP multiplies + PE accumulation matmuls
    (constant identity stationary, 26 accumulating steps).
  - per-cell expert matmuls run in "block-T" layout (features on partitions,
    32-cell blocks) with 4x block-diagonal weights; biases become
    per-partition ACT bias vectors.
"""

import numpy as np
import ml_dtypes
from contextlib import ExitStack

import concourse.bass as bass
import concourse.bacc as bacc
import concourse.tile as tile
import concourse.mybir as mybir

B, K, D, NH = 262144, 26, 32, 32
N_CORES = 8
BS = B // N_CORES  # 32768 cells per core
CT = 128           # cells per tile
N_STEPS = 3
DT_STEP = 1.0 / N_STEPS

dt = mybir.dt
bf16 = ml_dtypes.bfloat16
AF = mybir.ActivationFunctionType
ALU = mybir.AluOpType

# column offsets into the packed weight-constant dram tensor [128, WC_COLS]
_WSLOTS = ["W4msg", "Wl_t", "Wl_b", "Wu_t", "Wu_b", "Wc_t", "Wc_b",
           "Wg1_t", "Wg1_b", "I128"]
WC_COLS = 128 * len(_WSLOTS) + 96  # + Wg2rep [128, 96]
BC_COLS = 8  # f32 bias consts


def _wslot(name):
    return 128 * _WSLOTS.index(name)


def build_program(bs=BS, ct=CT):
    """Builds the per-core Bass program (SPMD; all cores identical)."""
    nt = bs // ct
    nc = bacc.Bacc("TRN2", target_bir_lowering=False, debug=False,
                   num_devices=N_CORES)

    a_nbn = nc.dram_tensor("nbn", [bs, K * D], dt.bfloat16, kind="ExternalInput").ap()
    a_csn = nc.dram_tensor("csn", [bs, D], dt.float32, kind="ExternalInput").ap()
    a_cst = nc.dram_tensor("cst", [128, nt * D], dt.bfloat16, kind="ExternalInput").ap()
    a_msk = nc.dram_tensor("msk", [bs, 80], dt.bfloat16, kind="ExternalInput").ap()
    a_scl = nc.dram_tensor("scl", [bs, 4], dt.float32, kind="ExternalInput").ap()
    a_wc = nc.dram_tensor("wc", [128, WC_COLS], dt.bfloat16, kind="ExternalInput").ap()
    a_bc = nc.dram_tensor("bc", [128, BC_COLS], dt.float32, kind="ExternalInput").ap()
    a_out = nc.dram_tensor("out", [bs, D], dt.float32, kind="ExternalOutput").ap()

    with tile.TileContext(nc) as tc:
        _body(tc, a_nbn, a_csn, a_cst, a_msk, a_scl, a_wc, a_bc, a_out, bs, ct, nt)
    nc.compile()
    return nc


def _body(tc, a_nbn, a_csn, a_cst, a_msk, a_scl, a_wc, a_bc, a_out, bs, ct, nt):
    nc = tc.nc
    FR = K * D  # 832

    with ExitStack() as ctx:
        cpool = ctx.enter_context(tc.tile_pool(name="const", bufs=1))
        pin = ctx.enter_context(tc.tile_pool(name="in", bufs=3))
        psml = ctx.enter_context(tc.tile_pool(name="small", bufs=3))
        pbig = ctx.enter_context(tc.tile_pool(name="big", bufs=2))
        pps_m = ctx.enter_context(tc.tile_pool(name="psm", bufs=2, space="PSUM"))
        pps_s = ctx.enter_context(tc.tile_pool(name="pss", bufs=2, space="PSUM"))
        pps_d = ctx.enter_context(tc.tile_pool(name="psd", bufs=2, space="PSUM"))

        wc = cpool.tile([128, WC_COLS], dt.bfloat16, tag="wc")
        nc.sync.dma_start(wc[:], a_wc)
        bc = cpool.tile([128, BC_COLS], dt.float32, tag="bc")
        nc.sync.dma_start(bc[:], a_bc)

        def W(name):
            return wc[:, _wslot(name): _wslot(name) + 128]

        w2rep = wc[:, 128 * len(_WSLOTS): 128 * len(_WSLOTS) + 96]
        b_msg4 = bc[:, 0:1]
        b_loc4 = bc[:, 1:2]
        b_upd4 = bc[:, 2:3]
        b_cnf4 = bc[:, 3:4]
        b_g14 = bc[:, 4:5]
        bg2rep = bc[:, 5:8]  # [128, 3] replicated b_g2 row

        for t in range(nt):
            r0 = t * ct
            rows = slice(r0, r0 + ct)

            # ---- loads ----
            nb = pin.tile([128, FR], dt.bfloat16, tag="nb")
            nc.sync.dma_start(nb[:], a_nbn[rows, :])
            csn = psml.tile([128, D], dt.float32, tag="csn")
            nc.sync.dma_start(csn[:], a_csn[rows, :])
            cst = psml.tile([128, D], dt.bfloat16, tag="cst")
            nc.sync.dma_start(cst[:], a_cst[:, t * D:(t + 1) * D])
            msk = psml.tile([128, 80], dt.bfloat16, tag="msk")
            nc.sync.dma_start(msk[:], a_msk[rows, :])
            scl = psml.tile([128, 4], dt.float32, tag="scl")
            nc.sync.dma_start(scl[:], a_scl[rows, :])

            nb3 = nb[:].rearrange("p (k d) -> p k d", k=K)

            # ---- transpose for the message matmul ----
            nbT = pbig.tile([128, FR], dt.bfloat16, tag="nbT")
            nc.vector.transpose(nbT[:], nb[:])

            # ---- msgs = tanh(nb @ W_msg + b_msg), transposed layout ----
            ps_m0 = pps_m.tile([128, 416], dt.float32, tag="psm0")
            ps_m1 = pps_m.tile([128, 416], dt.float32, tag="psm1")
            nc.tensor.matmul(ps_m0[:], W("W4msg"), nbT[:, 0:416], start=True, stop=True)
            nc.tensor.matmul(ps_m1[:], W("W4msg"), nbT[:, 416:832], start=True, stop=True)
            msgsT = pbig.tile([128, FR], dt.bfloat16, tag="msgsT")
            nc.scalar.activation(msgsT[:, 0:416], ps_m0[:], AF.Tanh, bias=b_msg4, scale=1.0)
            nc.scalar.activation(msgsT[:, 416:832], ps_m1[:], AF.Tanh, bias=b_msg4, scale=1.0)

            # back to natural layout for the masked k-sum
            msgs_nat = pbig.tile([128, FR], dt.bfloat16, tag="msgsnat")
            nc.vector.transpose(msgs_nat[:], msgsT[:])
            msgs_nat3 = msgs_nat[:].rearrange("p (k d) -> p k d", k=K)

            # ---- masked products (broadcast-AP multiplies) ----
            def bmask(c0):
                return msk[:, c0:c0 + K].unsqueeze(2).to_broadcast((128, K, D))

            prodF = pbig.tile([128, FR], dt.bfloat16, tag="prodF")
            nc.vector.tensor_tensor(
                out=prodF[:].rearrange("p (k d) -> p k d", k=K),
                in0=msgs_nat3, in1=bmask(52), op=ALU.mult)
            prodA = pbig.tile([128, FR], dt.bfloat16, tag="prodA")
            nc.vector.tensor_tensor(
                out=prodA[:].rearrange("p (k d) -> p k d", k=K),
                in0=nb3, in1=bmask(0), op=ALU.mult)
            prodB = pbig.tile([128, FR], dt.bfloat16, tag="prodB")
            nc.vector.tensor_tensor(
                out=prodB[:].rearrange("p (k d) -> p k d", k=K),
                in0=nb3, in1=bmask(26), op=ALU.mult)

            # ---- k-sums via PE accumulation (identity stationary) ----
            # ps_sums columns: S0 @0, A @32, B @64, agg @96
            ps_sums = pps_s.tile([128, 128], dt.float32, tag="sums")
            srcs = [nb3, prodA[:].rearrange("p (k d) -> p k d", k=K),
                    prodB[:].rearrange("p (k d) -> p k d", k=K),
                    prodF[:].rearrange("p (k d) -> p k d", k=K)]
            for j, src in enumerate(srcs):
                for b in range(K):
                    nc.tensor.matmul(ps_sums[:, 32 * j:32 * j + 32], W("I128"),
                                     src[:, b, :], start=(b == 0), stop=(b == K - 1))

            S0 = ps_sums[:, 0:32]
            SA = ps_sums[:, 32:64]
            SB = ps_sums[:, 64:96]
            Sagg = ps_sums[:, 96:128]

            # ---- means (natural, f32) ----
            S0sb = psml.tile([128, D], dt.float32, tag="S0sb")
            nc.vector.tensor_copy(S0sb[:], S0)
            tmp_loc = psml.tile([128, D], dt.float32, tag="tmploc")
            nc.vector.tensor_tensor(out=tmp_loc[:], in0=S0sb[:], in1=SA, op=ALU.subtract)
            mean_loc = psml.tile([128, D], dt.bfloat16, tag="mloc")
            nc.vector.tensor_scalar(out=mean_loc[:], in0=tmp_loc[:],
                                    scalar1=scl[:, 0:1], scalar2=None, op0=ALU.mult)
            mean_dis = psml.tile([128, D], dt.bfloat16, tag="mdis")
            nc.vector.tensor_scalar(out=mean_dis[:], in0=SB,
                                    scalar1=scl[:, 1:2], scalar2=None, op0=ALU.mult)
            S0b16 = psml.tile([128, D], dt.bfloat16, tag="S0b16")
            nc.vector.tensor_copy(S0b16[:], S0sb[:])
            agg16 = psml.tile([128, D], dt.bfloat16, tag="agg16")
            nc.vector.tensor_copy(agg16[:], Sagg)

            # ---- tiny transposes into block-T layout (bf16 operands) ----
            mlT = psml.tile([128, D], dt.bfloat16, tag="mlT")
            nc.vector.transpose(mlT[:], mean_loc[:])
            mdT = psml.tile([128, D], dt.bfloat16, tag="mdT")
            nc.vector.transpose(mdT[:], mean_dis[:])
            mnT = psml.tile([128, D], dt.bfloat16, tag="mnT")
            nc.vector.transpose(mnT[:], S0b16[:])  # 1/K folded into Wg1_b on host
            aggT = psml.tile([128, D], dt.bfloat16, tag="aggT")
            nc.vector.transpose(aggT[:], agg16[:])
            xT = psml.tile([128, D], dt.float32, tag="xT")
            nc.vector.transpose(xT[:], csn[:])

            # ---- experts (block-T, PE + ACT) ----
            ps_dn = pps_d.tile([128, 192], dt.float32, tag="dn")

            nc.tensor.matmul(ps_dn[:, 0:32], W("Wl_t"), cst[:], start=True, stop=False)
            nc.tensor.matmul(ps_dn[:, 0:32], W("Wl_b"), mlT[:], start=False, stop=True)
            localT = psml.tile([128, D], dt.float32, tag="localT")
            nc.scalar.activation(localT[:], ps_dn[:, 0:32], AF.Tanh, bias=b_loc4, scale=1.0)

            nc.tensor.matmul(ps_dn[:, 32:64], W("Wu_t"), cst[:], start=True, stop=False)
            nc.tensor.matmul(ps_dn[:, 32:64], W("Wu_b"), aggT[:], start=False, stop=True)
            zT = psml.tile([128, D], dt.float32, tag="zT")
            nc.scalar.activation(zT[:], ps_dn[:, 32:64], AF.Sigmoid, bias=b_upd4, scale=1.0)

            nc.tensor.matmul(ps_dn[:, 64:96], W("Wg1_t"), cst[:], start=True, stop=False)
            nc.tensor.matmul(ps_dn[:, 64:96], W("Wg1_b"), mnT[:], start=False, stop=True)
            hT = psml.tile([128, D], dt.bfloat16, tag="hT")
            nc.scalar.activation(hT[:], ps_dn[:, 64:96], AF.Relu, bias=b_g14, scale=1.0)

            # CNF Euler steps (x kept f32, bf16 copies feed the PE)
            xcur = xT
            xbf = cst  # step-1 moving operand is exactly csT (bf16)
            for s in range(N_STEPS):
                nc.tensor.matmul(ps_dn[:, 128:160], W("Wc_t"), xbf[:], start=True, stop=False)
                nc.tensor.matmul(ps_dn[:, 128:160], W("Wc_b"), mdT[:], start=False, stop=True)
                vb = psml.tile([128, D], dt.float32, tag=f"vb{s}")
                nc.scalar.activation(vb[:], ps_dn[:, 128:160], AF.Tanh, bias=b_cnf4, scale=1.0)
                xnew = psml.tile([128, D], dt.float32, tag=f"xn{s}")
                nc.vector.scalar_tensor_tensor(out=xnew[:], in0=vb[:], scalar=DT_STEP,
                                               in1=xcur[:], op0=ALU.mult, op1=ALU.add)
                xcur = xnew
                if s < N_STEPS - 1:
                    xb2 = psml.tile([128, D], dt.bfloat16, tag=f"xb{s}")
                    nc.scalar.copy(xb2[:], xnew[:])
                    xbf = xb2

            # ---- gating (natural layout) ----
            h_nat = psml.tile([128, D], dt.bfloat16, tag="hnat")
            nc.vector.transpose(h_nat[:], hT[:])
            lg = psml.tile([128, 4], dt.float32, tag="lg")
            for g in range(3):
                gp = psml.tile([128, D], dt.bfloat16, tag="gp")
                nc.vector.tensor_tensor(out=gp[:], in0=h_nat[:],
                                        in1=w2rep[:, 32 * g:32 * g + 32], op=ALU.mult)
                nc.vector.tensor_reduce(out=lg[:, g:g + 1], in_=gp[:],
                                        axis=mybir.AxisListType.X, op=ALU.add)
            lgb = psml.tile([128, 3], dt.float32, tag="lgb")
            nc.vector.tensor_tensor(out=lgb[:], in0=lg[:, 0:3], in1=bg2rep, op=ALU.add)
            eg = psml.tile([128, 3], dt.float32, tag="eg")
            nc.scalar.activation(eg[:], lgb[:], AF.Exp)
            sg = psml.tile([128, 1], dt.float32, tag="sg")
            nc.vector.tensor_reduce(out=sg[:], in_=eg[:], axis=mybir.AxisListType.X, op=ALU.add)
            rinv = psml.tile([128, 1], dt.float32, tag="rinv")
            nc.vector.reciprocal(rinv[:], sg[:])
            gts = psml.tile([128, 3], dt.float32, tag="gts")
            nc.vector.tensor_scalar(out=gts[:], in0=eg[:], scalar1=rinv[:],
                                    scalar2=None, op0=ALU.mult)

            # ---- func expert combine (natural) ----
            tanh_agg = psml.tile([128, D], dt.float32, tag="tagg")
            nc.scalar.activation(tanh_agg[:], Sagg, AF.Tanh)
            z_nat = psml.tile([128, D], dt.float32, tag="znat")
            nc.vector.transpose(z_nat[:], zT[:])
            d2 = psml.tile([128, D], dt.float32, tag="d2")
            nc.vector.tensor_tensor(out=d2[:], in0=tanh_agg[:], in1=csn[:], op=ALU.subtract)
            f1 = psml.tile([128, D], dt.float32, tag="f1")
            nc.vector.tensor_tensor(out=f1[:], in0=z_nat[:], in1=d2[:], op=ALU.mult)
            func_nat = psml.tile([128, D], dt.float32, tag="func")
            nc.vector.tensor_tensor(out=func_nat[:], in0=f1[:], in1=csn[:], op=ALU.add)

            # ---- experts back to natural + weighted combine ----
            local_nat = psml.tile([128, D], dt.float32, tag="locnat")
            nc.vector.transpose(local_nat[:], localT[:])
            dist_nat = psml.tile([128, D], dt.float32, tag="distnat")
            nc.vector.transpose(dist_nat[:], xcur[:])

            acc1 = psml.tile([128, D], dt.float32, tag="acc1")
            nc.vector.tensor_scalar(out=acc1[:], in0=local_nat[:],
                                    scalar1=gts[:, 0:1], scalar2=None, op0=ALU.mult)
            acc2 = psml.tile([128, D], dt.float32, tag="acc2")
            nc.vector.scalar_tensor_tensor(out=acc2[:], in0=func_nat[:], scalar=gts[:, 1:2],
                                           in1=acc1[:], op0=ALU.mult, op1=ALU.add)
            acc3 = psml.tile([128, D], dt.float32, tag="acc3")
            nc.vector.scalar_tensor_tensor(out=acc3[:], in0=dist_nat[:], scalar=gts[:, 2:3],
                                           in1=acc2[:], op0=ALU.mult, op1=ALU.add)

            nc.sync.dma_start(a_out[rows, :], acc3[:])


# ---------------------------------------------------------------------------
# host staging
# ---------------------------------------------------------------------------

def stage_inputs(inputs, bs=BS, ct=CT):
    """Returns (in_maps, weights_dict) for run_bass_kernel_spmd."""
    nt = bs // ct
    cs = np.asarray(inputs["current_state"], np.float32)
    nb = np.asarray(inputs["neighbor_states"], np.float32)
    tiers = np.asarray(inputs["tier_ids"], np.int32)

    f32 = np.float32
    W_local = np.asarray(inputs["W_local"], f32)
    W_msg = np.asarray(inputs["W_msg"], f32)
    W_upd = np.asarray(inputs["W_upd"], f32)
    W_cnf = np.asarray(inputs["W_cnf"], f32)
    W_g1 = np.asarray(inputs["W_g1"], f32)
    W_g2 = np.asarray(inputs["W_g2"], f32)
    b_msg = np.asarray(inputs["b_msg"], f32)
    b_local = np.asarray(inputs["b_local"], f32)
    b_upd = np.asarray(inputs["b_upd"], f32)
    b_cnf = np.asarray(inputs["b_cnf"], f32)
    b_g1 = np.asarray(inputs["b_g1"], f32)
    b_g2 = np.asarray(inputs["b_g2"], f32)

    eye4 = np.eye(4, dtype=f32)

    def kron4(w):
        return np.kron(eye4, w)

    wparts = {
        "W4msg": kron4(W_msg),
        "Wl_t": kron4(W_local[:D]), "Wl_b": kron4(W_local[D:]),
        "Wu_t": kron4(W_upd[:D]), "Wu_b": kron4(W_upd[D:]),
        "Wc_t": kron4(W_cnf[:D]), "Wc_b": kron4(W_cnf[D:]),
        "Wg1_t": kron4(W_g1[:D]), "Wg1_b": kron4(W_g1[D:] / K),
        "I128": np.eye(128, dtype=f32),
    }
    wc = np.zeros((128, WC_COLS), f32)
    for name in _WSLOTS:
        wc[:, _wslot(name):_wslot(name) + 128] = wparts[name]
    for g in range(3):
        wc[:, 128 * len(_WSLOTS) + 32 * g: 128 * len(_WSLOTS) + 32 * g + 32] = W_g2[:, g][None, :]
    wc = wc.astype(bf16)

    bcq = np.zeros((128, BC_COLS), f32)
    bcq[:, 0] = np.tile(b_msg, 4)
    bcq[:, 1] = np.tile(b_local, 4)
    bcq[:, 2] = np.tile(b_upd, 4)
    bcq[:, 3] = np.tile(b_cnf, 4)
    bcq[:, 4] = np.tile(b_g1, 4)
    bcq[:, 5:8] = b_g2[None, :]

    in_maps = []
    for c in range(N_CORES):
        rs = slice(c * bs, (c + 1) * bs)
        nb_c = nb[rs]
        cs_c = cs[rs]
        tr_c = tiers[rs]

        nbn = nb_c.reshape(bs, K * D).astype(bf16)

        cs4 = cs_c.reshape(nt, 4, 32, D).transpose(0, 1, 3, 2)  # [t, a, d, c]
        cst = cs4.reshape(nt, 128, 32).transpose(1, 0, 2).reshape(128, nt * 32).astype(bf16)

        mA = (tr_c >= 1)
        mB = (tr_c == 2)
        m1 = (tr_c == 1)
        cnt0 = (tr_c == 0).sum(-1).astype(f32)
        cnt1 = m1.sum(-1).astype(f32)
        cnt2 = mB.sum(-1).astype(f32)
        wfun = m1.astype(f32) / np.maximum(cnt1, 1.0)[:, None]
        msk = np.zeros((bs, 80), f32)
        msk[:, 0:K] = mA
        msk[:, 26:26 + K] = mB
        msk[:, 52:52 + K] = wfun
        msk = msk.astype(bf16)

        scl = np.zeros((bs, 4), f32)
        scl[:, 0] = 1.0 / np.maximum(cnt0, 1.0)
        scl[:, 1] = 1.0 / np.maximum(cnt2, 1.0)

        in_maps.append({
            "nbn": nbn, "csn": cs_c.astype(f32), "cst": cst,
            "msk": msk, "scl": scl, "wc": wc, "bc": bcq,
        })
    return in_maps


_PROGRAM_CACHE = {}


def kernel(**inputs):
    from concourse.bass_utils import run_bass_kernel_spmd

    key = (BS, CT)
    if key not in _PROGRAM_CACHE:
        _PROGRAM_CACHE[key] = build_program(BS, CT)
    nc = _PROGRAM_CACHE[key]

    in_maps = stage_inputs(inputs, BS, CT)
    res = run_bass_kernel_spmd(nc, in_maps, core_ids=list(range(N_CORES)))
    out = np.concatenate([r["out"] for r in res.results], axis=0)
    return out.astype(np.float32)



# revision 7
# speedup vs baseline: 3.1920x; 3.1920x over previous
"""Trainium2 Bass kernel for nn_MoEConnectionProcessor (v2: all-blockT).

Strategy
--------
Data-parallel over 8 cores (32768 cells each). Per core, cells are processed
in super-tiles (ST) of 2048 cells laid out "blockT": SBUF partition =
(g, d) with g = cell-subgroup (4 of 32 cells within a 128-cell tile),
d = feature; free axis = (t, c) = (tile-in-ST, cell-in-subgroup) = 512 cols.

The host pre-sorts each cell's 26 neighbors by tier and stages THREE
premasked copies of neighbor_states (tier-0/1/2 * nb), truncated to the
global max per-tier count W_t (~21), already in blockT with j (neighbor
slot) outermost. Because the masks are 0/1 and the tier classes partition
the neighbors:

  - S_t = sum_k m_t*nb   becomes an UNMASKED PE accumulation chain over j
    (identity stationary, premasked zeros contribute nothing) -> no DVE
    mask products, no transposes, no reduces.
  - tanh(m1 * msg) = m1 * tanh(msg) (b_msg == 0 per spec), so the
    functional expert's masked message sum is: matmul kron(I4, W_msg) per
    j-slot -> ACT tanh -> PE accumulation chain.
  - S0 = S_t0 + S_t1 + S_t2 (two cheap adds), loc_mean = S_t0/cnt0, etc.

All experts, gating, CNF steps and the final combine run in blockT
(biases are per-partition there). Gates ([12, 512] = (g, expert) rows)
are broadcast to 128 partitions with tiny scatter matmuls. Output stays
blockT in DRAM; the host inverse-permutes.

sigmoid(x) = 0.5*tanh(0.5x) + 0.5 and relu on DVE keep every ACT function
in one activation-table set (no ACT_TABLE_LOAD churn).
"""

import numpy as np
import ml_dtypes
from contextlib import ExitStack

import concourse.bass as bass
import concourse.bacc as bacc
import concourse.tile as tile
import concourse.mybir as mybir

B, K, D, NH = 262144, 26, 32, 32
N_CORES = 8
BS = B // N_CORES          # 32768 cells per core
ST = 2048                  # cells per super-tile
NT = BS // ST              # 16 super-tiles per core
TPS = ST // 128            # 16 tiles of 128 cells per super-tile
SC = TPS * 32              # 512 free columns per super-tile (t, c)
N_STEPS = 3
DT_STEP = 1.0 / N_STEPS

dt = mybir.dt
bf16 = ml_dtypes.bfloat16
f8e4 = ml_dtypes.float8_e4m3
AF = mybir.ActivationFunctionType
ALU = mybir.AluOpType

# staged dtype of the three big premasked neighbor copies
STAGE_DT = dt.bfloat16
STAGE_NP = bf16

# stationary slots in the packed weight tensor [128, n*128 + 12 + 4 + 3*128 + 12]
_WSLOTS = ["I128", "W4msg", "Wl_t", "Wl_b", "Wu_t", "Wu_b", "Wc_t", "Wc_b",
           "Wg1_t", "Wg1_b"]
# extra (non-128-wide) stationaries appended after the slots:
#   kron(I4, W_g2):      [128, 12]
#   ones_sum:            [12, 4]   (pad part-dim to 12 rows used)
#   recip bcast (f32):   [4, 12]
#   gate scatter e=0..2: [12, 128] each
EX_G2 = 128 * len(_WSLOTS)
EX_ONES = EX_G2 + 12
EX_SCAT = EX_ONES + 4          # 3x128 bf16 scatter
WC_COLS = EX_SCAT + 3 * 128
WF_COLS = 12                   # f32 tensor: recip-bcast [4, 12]
BC_COLS = 8                    # f32 biases


def _wslot(name):
    return 128 * _WSLOTS.index(name)


def build_program(w0, w1, w2):
    nc = bacc.Bacc("TRN2", target_bir_lowering=False, debug=False,
                   num_devices=N_CORES)

    a_m0 = nc.dram_tensor("m0", [128, NT * w0 * SC], STAGE_DT, kind="ExternalInput").ap()
    a_m1 = nc.dram_tensor("m1", [128, NT * w1 * SC], STAGE_DT, kind="ExternalInput").ap()
    a_m2 = nc.dram_tensor("m2", [128, NT * w2 * SC], STAGE_DT, kind="ExternalInput").ap()
    a_cst = nc.dram_tensor("cst", [128, NT * SC], dt.bfloat16, kind="ExternalInput").ap()
    a_icn = nc.dram_tensor("icn", [128, NT * 3 * SC], dt.bfloat16, kind="ExternalInput").ap()
    a_wc = nc.dram_tensor("wc", [128, WC_COLS], dt.bfloat16, kind="ExternalInput").ap()
    a_wf = nc.dram_tensor("wf", [4, WF_COLS], dt.float32, kind="ExternalInput").ap()
    a_bc = nc.dram_tensor("bc", [128, BC_COLS], dt.float32, kind="ExternalInput").ap()
    a_out = nc.dram_tensor("out", [128, NT * SC], dt.float32, kind="ExternalOutput").ap()

    with tile.TileContext(nc) as tc:
        _body(tc, a_m0, a_m1, a_m2, a_cst, a_icn, a_wc, a_wf, a_bc, a_out,
              w0, w1, w2)
    nc.compile()
    return nc


def _body(tc, a_m0, a_m1, a_m2, a_cst, a_icn, a_wc, a_wf, a_bc, a_out,
          w0, w1, w2):
    nc = tc.nc

    with ExitStack() as ctx:
        cpool = ctx.enter_context(tc.tile_pool(name="const", bufs=1))
        pin0 = ctx.enter_context(tc.tile_pool(name="in0", bufs=2))
        pin1 = ctx.enter_context(tc.tile_pool(name="in1", bufs=2))
        pin2 = ctx.enter_context(tc.tile_pool(name="in2", bufs=2))
        pinc = ctx.enter_context(tc.tile_pool(name="inc", bufs=2))
        ptnh = ctx.enter_context(tc.tile_pool(name="tnh", bufs=1))
        psml = ctx.enter_context(tc.tile_pool(name="sml", bufs=1))
        pout = ctx.enter_context(tc.tile_pool(name="out", bufs=2))
        # PSUM: chains 4 banks (bufs=1) + msgs 2 banks + experts 2 banks = 8
        pps_ch = ctx.enter_context(tc.tile_pool(name="pch", bufs=1, space="PSUM"))
        pps_m = ctx.enter_context(tc.tile_pool(name="pm", bufs=2, space="PSUM"))
        pps_e = ctx.enter_context(tc.tile_pool(name="pe", bufs=2, space="PSUM"))

        wc = cpool.tile([128, WC_COLS], dt.bfloat16, tag="wc")
        nc.sync.dma_start(wc[:], a_wc)
        wf = cpool.tile([4, WF_COLS], dt.float32, tag="wf")
        nc.sync.dma_start(wf[:], a_wf)
        bc = cpool.tile([128, BC_COLS], dt.float32, tag="bc")
        nc.sync.dma_start(bc[:], a_bc)

        def W(name):
            return wc[:, _wslot(name): _wslot(name) + 128]

        kron_g2 = wc[:, EX_G2:EX_G2 + 12]
        ones_sum = wc[0:12, EX_ONES:EX_ONES + 4]
        rb_f32 = wf[0:4, 0:12]
        scat = [wc[0:12, EX_SCAT + 128 * e: EX_SCAT + 128 * (e + 1)]
                for e in range(3)]

        b_loc4 = bc[:, 1:2]
        b_updh = bc[:, 2:3]   # 0.5 * b_upd, for sigmoid-via-tanh
        b_cnf4 = bc[:, 3:4]
        b_g14 = bc[:, 4:5]
        b_g2r = bc[0:12, 5:6]  # b_g2 on (g,e) rows 0..11
        b_msg4 = bc[:, 0:1]

        for i in range(NT):
            m0 = pin0.tile([128, w0 * SC], STAGE_DT, tag="m0")
            nc.sync.dma_start(m0[:], a_m0[:, i * w0 * SC:(i + 1) * w0 * SC])
            m1 = pin1.tile([128, w1 * SC], STAGE_DT, tag="m1")
            nc.sync.dma_start(m1[:], a_m1[:, i * w1 * SC:(i + 1) * w1 * SC])
            m2 = pin2.tile([128, w2 * SC], STAGE_DT, tag="m2")
            nc.sync.dma_start(m2[:], a_m2[:, i * w2 * SC:(i + 1) * w2 * SC])
            cst = pinc.tile([128, SC], dt.bfloat16, tag="cst")
            nc.sync.dma_start(cst[:], a_cst[:, i * SC:(i + 1) * SC])
            icn = pinc.tile([128, 3 * SC], dt.bfloat16, tag="icn")
            nc.sync.dma_start(icn[:], a_icn[:, i * 3 * SC:(i + 1) * 3 * SC])
            inv0 = icn[:, 0:SC]
            inv1 = icn[:, SC:2 * SC]
            inv2 = icn[:, 2 * SC:3 * SC]

            # ---- chain psums: St0 | St1 | St2 | agg (4 banks) ----
            pch = pps_ch.tile([128, 4 * SC], dt.float32, tag="ch")
            pSt0 = pch[:, 0:SC]
            pSt1 = pch[:, SC:2 * SC]
            pSt2 = pch[:, 2 * SC:3 * SC]
            pAgg = pch[:, 3 * SC:4 * SC]

            # raw tier-sum accumulation chains (identity stationary)
            for j in range(w0):
                nc.tensor.matmul(pSt0, W("I128"), m0[:, j * SC:(j + 1) * SC],
                                 start=(j == 0), stop=(j == w0 - 1))
            for j in range(w1):
                nc.tensor.matmul(pSt1, W("I128"), m1[:, j * SC:(j + 1) * SC],
                                 start=(j == 0), stop=(j == w1 - 1))
            for j in range(w2):
                nc.tensor.matmul(pSt2, W("I128"), m2[:, j * SC:(j + 1) * SC],
                                 start=(j == 0), stop=(j == w2 - 1))

            # ---- msgs: per-j matmul + tanh into SBUF, then accum chain ----
            tnh = ptnh.tile([128, w1 * SC], dt.bfloat16, tag="tnh")
            for j in range(w1):
                pm = pps_m.tile([128, SC], dt.float32, tag=f"pm")
                nc.tensor.matmul(pm[:], W("W4msg"), m1[:, j * SC:(j + 1) * SC],
                                 start=True, stop=True)
                nc.scalar.activation(tnh[:, j * SC:(j + 1) * SC], pm[:],
                                     AF.Tanh, bias=b_msg4, scale=1.0)
            for j in range(w1):
                nc.tensor.matmul(pAgg, W("I128"), tnh[:, j * SC:(j + 1) * SC],
                                 start=(j == 0), stop=(j == w1 - 1))

            # ---- means / S0 (blockT, bf16 operands for expert matmuls) ----
            mloc = psml.tile([128, SC], dt.bfloat16, tag="mloc")
            nc.vector.tensor_tensor(out=mloc[:], in0=pSt0, in1=inv0, op=ALU.mult)
            mdis = psml.tile([128, SC], dt.bfloat16, tag="mdis")
            nc.vector.tensor_tensor(out=mdis[:], in0=pSt2, in1=inv2, op=ALU.mult)
            aggb = psml.tile([128, SC], dt.bfloat16, tag="aggb")
            nc.vector.tensor_tensor(out=aggb[:], in0=pAgg, in1=inv1, op=ALU.mult)
            st1c = psml.tile([128, SC], dt.bfloat16, tag="st1c")
            nc.scalar.copy(st1c[:], pSt1)
            s01 = psml.tile([128, SC], dt.bfloat16, tag="s01")
            nc.vector.tensor_tensor(out=s01[:], in0=pSt0, in1=st1c[:], op=ALU.add)
            s0 = psml.tile([128, SC], dt.bfloat16, tag="s0")
            nc.vector.tensor_tensor(out=s0[:], in0=pSt2, in1=s01[:], op=ALU.add)

            # ---- local expert: tanh([cs, loc_mean] @ W_local + b) ----
            pl = pps_e.tile([128, SC], dt.float32, tag="pe")
            nc.tensor.matmul(pl[:], W("Wl_t"), cst[:], start=True, stop=False)
            nc.tensor.matmul(pl[:], W("Wl_b"), mloc[:], start=False, stop=True)
            locb = psml.tile([128, SC], dt.bfloat16, tag="locb")
            nc.scalar.activation(locb[:], pl[:], AF.Tanh, bias=b_loc4, scale=1.0)

            # ---- func expert: z = sigmoid(u) = 0.5*tanh(0.5u + 0.5b) + 0.5
            pu = pps_e.tile([128, SC], dt.float32, tag="pe")
            nc.tensor.matmul(pu[:], W("Wu_t"), cst[:], start=True, stop=False)
            nc.tensor.matmul(pu[:], W("Wu_b"), aggb[:], start=False, stop=True)
            tu = psml.tile([128, SC], dt.bfloat16, tag="tu")
            nc.scalar.activation(tu[:], pu[:], AF.Tanh, bias=b_updh, scale=0.5)
            tagg = psml.tile([128, SC], dt.bfloat16, tag="tagg")
            nc.scalar.activation(tagg[:], aggb[:], AF.Tanh)
            d2 = psml.tile([128, SC], dt.bfloat16, tag="d2")
            nc.vector.tensor_tensor(out=d2[:], in0=tagg[:], in1=cst[:], op=ALU.subtract)
            e1 = psml.tile([128, SC], dt.bfloat16, tag="e1")
            nc.vector.scalar_tensor_tensor(out=e1[:], in0=tu[:], scalar=0.5,
                                           in1=d2[:], op0=ALU.mult, op1=ALU.mult)
            e2 = psml.tile([128, SC], dt.bfloat16, tag="e2")
            nc.vector.scalar_tensor_tensor(out=e2[:], in0=d2[:], scalar=0.5,
                                           in1=cst[:], op0=ALU.mult, op1=ALU.add)
            funcb = psml.tile([128, SC], dt.bfloat16, tag="funcb")
            nc.vector.tensor_tensor(out=funcb[:], in0=e1[:], in1=e2[:], op=ALU.add)

            # ---- distant expert: 3 Euler steps, x kept bf16 ----
            xb = cst
            for s in range(N_STEPS):
                pc = pps_e.tile([128, SC], dt.float32, tag="pe")
                nc.tensor.matmul(pc[:], W("Wc_t"), xb[:], start=True, stop=False)
                nc.tensor.matmul(pc[:], W("Wc_b"), mdis[:], start=False, stop=True)
                vb = psml.tile([128, SC], dt.bfloat16, tag=f"vb{s}")
                nc.scalar.activation(vb[:], pc[:], AF.Tanh, bias=b_cnf4, scale=1.0)
                xn = psml.tile([128, SC], dt.bfloat16, tag=f"xn{s}")
                nc.vector.scalar_tensor_tensor(out=xn[:], in0=vb[:], scalar=DT_STEP,
                                               in1=xb[:], op0=ALU.mult, op1=ALU.add)
                xb = xn

            # ---- gating ----
            pg = pps_e.tile([128, SC], dt.float32, tag="pe")
            nc.tensor.matmul(pg[:], W("Wg1_t"), cst[:], start=True, stop=False)
            nc.tensor.matmul(pg[:], W("Wg1_b"), s0[:], start=False, stop=True)
            hb = psml.tile([128, SC], dt.bfloat16, tag="hb")
            nc.vector.tensor_scalar(out=hb[:], in0=pg[:], scalar1=b_g14,
                                    scalar2=0.0, op0=ALU.add, op1=ALU.max)
            pl2 = pps_e.tile([128, SC], dt.float32, tag="pe")
            nc.tensor.matmul(pl2[0:12, :], kron_g2, hb[:], start=True, stop=True)
            eg = psml.tile([12, SC], dt.bfloat16, tag="eg")
            nc.scalar.activation(eg[:], pl2[0:12, :], AF.Exp, bias=b_g2r, scale=1.0)
            ps = pps_e.tile([128, SC], dt.float32, tag="pe")
            nc.tensor.matmul(ps[0:4, :], ones_sum, eg[:], start=True, stop=True)
            rec = psml.tile([4, SC], dt.float32, tag="rec")
            nc.vector.reciprocal(rec[:], ps[0:4, :])
            prb = pps_e.tile([128, SC], dt.float32, tag="pe")
            nc.tensor.matmul(prb[0:12, :], rb_f32, rec[:], start=True, stop=True)
            gts = psml.tile([12, SC], dt.bfloat16, tag="gts")
            nc.vector.tensor_tensor(out=gts[:], in0=eg[:], in1=prb[0:12, :], op=ALU.mult)

            # gate broadcast (12 -> 128 partitions) + weighted combine.
            # Reuses the chain psum banks (fully consumed by the means above).
            pge = pch
            for e in range(3):
                nc.tensor.matmul(pge[:, e * SC:(e + 1) * SC], scat[e], gts[:],
                                 start=True, stop=True)
            a1 = psml.tile([128, SC], dt.bfloat16, tag="a1")
            nc.vector.tensor_tensor(out=a1[:], in0=pge[:, 0:SC], in1=locb[:], op=ALU.mult)
            a2 = psml.tile([128, SC], dt.bfloat16, tag="a2")
            nc.vector.tensor_tensor(out=a2[:], in0=pge[:, SC:2 * SC], in1=funcb[:], op=ALU.mult)
            a3 = psml.tile([128, SC], dt.bfloat16, tag="a3")
            nc.vector.tensor_tensor(out=a3[:], in0=pge[:, 2 * SC:3 * SC], in1=xb[:], op=ALU.mult)
            a12 = psml.tile([128, SC], dt.bfloat16, tag="a12")
            nc.vector.tensor_tensor(out=a12[:], in0=a1[:], in1=a2[:], op=ALU.add)
            outb = pout.tile([128, SC], dt.float32, tag="outb")
            nc.vector.tensor_tensor(out=outb[:], in0=a12[:], in1=a3[:], op=ALU.add)

            nc.sync.dma_start(a_out[:, i * SC:(i + 1) * SC], outb[:])


# ---------------------------------------------------------------------------
# host staging
# ---------------------------------------------------------------------------

def _to_blockT(arr_bsd):
    """[bs, d] (d == 32) -> blockT [128, NT*SC]: partition = g*32+d,
    cols = (i, t, c)."""
    bs, d = arr_bsd.shape
    a = arr_bsd.reshape(NT, TPS, 4, 32, d)           # [i, t, g, c, d]
    a = a.transpose(2, 4, 0, 1, 3)                   # [g, d, i, t, c]
    return np.ascontiguousarray(a.reshape(128, NT * SC))


def _nb_blockT(nb_sel):
    """[bs, w, 32] premasked sorted neighbors -> [128, NT*w*SC]:
    partition = g*32+d, cols = (i, j, t, c)."""
    bs, w, d = nb_sel.shape
    a = nb_sel.reshape(NT, TPS, 4, 32, w, d)         # [i, t, g, c, j, d]
    a = a.transpose(2, 5, 0, 4, 1, 3)                # [g, d, i, j, t, c]
    return np.ascontiguousarray(a.reshape(128, NT * w * SC))


def _from_blockT(arr):
    """inverse of _to_blockT: [128, NT*SC] -> [bs, 32]."""
    a = arr.reshape(4, 32, NT, TPS, 32)              # [g, d, i, t, c]
    a = a.transpose(2, 3, 0, 4, 1)                   # [i, t, g, c, d]
    return np.ascontiguousarray(a.reshape(NT * ST, 32))


def stage_weights(inputs, widths):
    f32 = np.float32
    W_local = np.asarray(inputs["W_local"], f32)
    W_msg = np.asarray(inputs["W_msg"], f32)
    W_upd = np.asarray(inputs["W_upd"], f32)
    W_cnf = np.asarray(inputs["W_cnf"], f32)
    W_g1 = np.asarray(inputs["W_g1"], f32)
    W_g2 = np.asarray(inputs["W_g2"], f32)

    eye4 = np.eye(4, dtype=f32)

    def kron4(w):
        return np.kron(eye4, w)

    wparts = {
        "I128": np.eye(128, dtype=f32),
        "W4msg": kron4(W_msg),
        "Wl_t": kron4(W_local[:D]), "Wl_b": kron4(W_local[D:]),
        "Wu_t": kron4(W_upd[:D]), "Wu_b": kron4(W_upd[D:]),
        "Wc_t": kron4(W_cnf[:D]), "Wc_b": kron4(W_cnf[D:]),
        "Wg1_t": kron4(W_g1[:D]), "Wg1_b": kron4(W_g1[D:] / K),
    }
    wc = np.zeros((128, WC_COLS), f32)
    for name in _WSLOTS:
        wc[:, _wslot(name):_wslot(name) + 128] = wparts[name]
    # kron(I4, W_g2): [128, 12]
    for g in range(4):
        wc[32 * g:32 * (g + 1), EX_G2 + 3 * g:EX_G2 + 3 * (g + 1)] = W_g2
    # ones_sum [12, 4]: row (g,e) -> col g
    for g in range(4):
        for e in range(3):
            wc[3 * g + e, EX_ONES + g] = 1.0
    # gate scatter: e fixed: [12, 128]: row (g,e') -> cols (g, d) if e'==e
    for e in range(3):
        for g in range(4):
            wc[3 * g + e, EX_SCAT + 128 * e + 32 * g:
               EX_SCAT + 128 * e + 32 * (g + 1)] = 1.0
    wc = wc.astype(bf16)

    wf = np.zeros((4, WF_COLS), f32)
    for g in range(4):
        wf[g, 3 * g:3 * (g + 1)] = 1.0   # recip bcast [4, 12]

    bcq = np.zeros((128, BC_COLS), f32)
    bcq[:, 0] = np.tile(np.asarray(inputs["b_msg"], f32), 4)
    bcq[:, 1] = np.tile(np.asarray(inputs["b_local"], f32), 4)
    bcq[:, 2] = 0.5 * np.tile(np.asarray(inputs["b_upd"], f32), 4)
    bcq[:, 3] = np.tile(np.asarray(inputs["b_cnf"], f32), 4)
    bcq[:, 4] = np.tile(np.asarray(inputs["b_g1"], f32), 4)
    b_g2 = np.asarray(inputs["b_g2"], f32)
    for g in range(4):
        bcq[3 * g:3 * (g + 1), 5] = b_g2
    return wc, wf, bcq


def stage_inputs(inputs):
    """Returns (in_maps, widths)."""
    f32 = np.float32
    cs = np.asarray(inputs["current_state"], f32)
    nb = np.asarray(inputs["neighbor_states"], f32)
    tiers = np.asarray(inputs["tier_ids"], np.int32)

    if np.any(np.asarray(inputs["b_msg"], f32) != 0.0):
        raise NotImplementedError("premask trick requires b_msg == 0")

    cnt = np.stack([(tiers == t).sum(-1) for t in range(3)], axis=1).astype(f32)  # [B, 3]
    widths = tuple(int(cnt[:, t].max()) for t in range(3))

    # per-tier sorted+premasked neighbor copies, truncated to widths
    copies = []
    for t in range(3):
        order = np.argsort(tiers != t, axis=1, kind="stable")[:, :widths[t]]
        sel = np.take_along_axis(nb, order[:, :, None], axis=1)
        msk = np.take_along_axis(tiers == t, order, axis=1)
        copies.append((sel * msk[:, :, None]).astype(STAGE_NP))

    inv = 1.0 / np.maximum(cnt, 1.0)       # [B, 3]

    wc, wf, bcq = stage_weights(inputs, widths)

    in_maps = []
    for c in range(N_CORES):
        rs = slice(c * BS, (c + 1) * BS)
        icn = np.empty((128, NT * 3 * SC), bf16)
        iv = [_to_blockT(np.repeat(inv[rs, t:t + 1], D, axis=1)) for t in range(3)]
        for i in range(NT):
            for t in range(3):
                icn[:, (3 * i + t) * SC:(3 * i + t + 1) * SC] = \
                    iv[t][:, i * SC:(i + 1) * SC]
        in_maps.append({
            "m0": _nb_blockT(copies[0][rs]),
            "m1": _nb_blockT(copies[1][rs]),
            "m2": _nb_blockT(copies[2][rs]),
            "cst": _to_blockT(cs[rs]).astype(bf16),
            "icn": icn,
            "wc": wc, "wf": wf, "bc": bcq,
        })
    return in_maps, widths


_PROGRAM_CACHE = {}


def kernel(**inputs):
    from concourse.bass_utils import run_bass_kernel_spmd

    in_maps, widths = stage_inputs(inputs)
    if widths not in _PROGRAM_CACHE:
        _PROGRAM_CACHE[widths] = build_program(*widths)
    nc = _PROGRAM_CACHE[widths]

    res = run_bass_kernel_spmd(nc, in_maps, core_ids=list(range(N_CORES)))
    out = np.concatenate([_from_blockT(np.asarray(r["out"], np.float32))
                          for r in res.results], axis=0)
    return out.astype(np.float32)


# revision 9
# speedup vs baseline: 3.5561x; 1.1140x over previous
"""Trainium2 Bass kernel for nn_MoEConnectionProcessor (v2: all-blockT).

Strategy
--------
Data-parallel over 8 cores (32768 cells each). Per core, cells are processed
in super-tiles (ST) of 2048 cells laid out "blockT": SBUF partition =
(g, d) with g = cell-subgroup (4 of 32 cells within a 128-cell tile),
d = feature; free axis = (t, c) = (tile-in-ST, cell-in-subgroup) = 512 cols.

The host pre-sorts each cell's 26 neighbors by tier and stages THREE
premasked copies of neighbor_states (tier-0/1/2 * nb), truncated to the
global max per-tier count W_t (~21), already in blockT with j (neighbor
slot) outermost. Because the masks are 0/1 and the tier classes partition
the neighbors:

  - S_t = sum_k m_t*nb   becomes an UNMASKED PE accumulation chain over j
    (identity stationary, premasked zeros contribute nothing) -> no DVE
    mask products, no transposes, no reduces.
  - tanh(m1 * msg) = m1 * tanh(msg) (b_msg == 0 per spec), so the
    functional expert's masked message sum is: matmul kron(I4, W_msg) per
    j-slot -> ACT tanh -> PE accumulation chain.
  - S0 = S_t0 + S_t1 + S_t2 (two cheap adds), loc_mean = S_t0/cnt0, etc.

All experts, gating, CNF steps and the final combine run in blockT
(biases are per-partition there). Gates ([12, 512] = (g, expert) rows)
are broadcast to 128 partitions with tiny scatter matmuls. Output stays
blockT in DRAM; the host inverse-permutes.

sigmoid(x) = 0.5*tanh(0.5x) + 0.5 and relu on DVE keep every ACT function
in one activation-table set (no ACT_TABLE_LOAD churn).
"""

import numpy as np
import ml_dtypes
from contextlib import ExitStack

import concourse.bass as bass
import concourse.bacc as bacc
import concourse.tile as tile
import concourse.mybir as mybir

B, K, D, NH = 262144, 26, 32, 32
N_CORES = 8
BS = B // N_CORES          # 32768 cells per core
ST = 2048                  # cells per super-tile
NT = BS // ST              # 16 super-tiles per core
TPS = ST // 128            # 16 tiles of 128 cells per super-tile
SC = TPS * 32              # 512 free columns per super-tile (t, c)
N_STEPS = 3
DT_STEP = 1.0 / N_STEPS

dt = mybir.dt
bf16 = ml_dtypes.bfloat16
f8e4 = ml_dtypes.float8_e4m3
AF = mybir.ActivationFunctionType
ALU = mybir.AluOpType

# staged dtype of the three big premasked neighbor copies
STAGE_DT = dt.bfloat16
STAGE_NP = bf16

# stationary slots in the packed weight tensor [128, n*128 + 12 + 4 + 3*128 + 12]
_WSLOTS = ["I128", "W4msg", "Wl_t", "Wl_b", "Wu_t", "Wu_b", "Wc_t", "Wc_b",
           "Wg1_t", "Wg1_b"]
# extra (non-128-wide) stationaries appended after the slots:
#   kron(I4, W_g2):      [128, 12]
#   ones_sum:            [12, 4]   (pad part-dim to 12 rows used)
#   recip bcast (f32):   [4, 12]
#   gate scatter e=0..2: [12, 128] each
EX_G2 = 128 * len(_WSLOTS)
EX_ONES = EX_G2 + 12
EX_SCAT = EX_ONES + 4          # 3x128 bf16 scatter
WC_COLS = EX_SCAT + 3 * 128
WF_COLS = 12                   # f32 tensor: recip-bcast [4, 12]
BC_COLS = 8                    # f32 biases


def _wslot(name):
    return 128 * _WSLOTS.index(name)


def build_program(w0, w1, w2):
    nc = bacc.Bacc("TRN2", target_bir_lowering=False, debug=False,
                   num_devices=N_CORES)

    a_m0 = nc.dram_tensor("m0", [128, NT * w0 * SC], STAGE_DT, kind="ExternalInput").ap()
    a_m1 = nc.dram_tensor("m1", [128, NT * w1 * SC], STAGE_DT, kind="ExternalInput").ap()
    a_m2 = nc.dram_tensor("m2", [128, NT * w2 * SC], STAGE_DT, kind="ExternalInput").ap()
    a_cst = nc.dram_tensor("cst", [128, NT * SC], dt.bfloat16, kind="ExternalInput").ap()
    a_icn = nc.dram_tensor("icn", [128, NT * 3 * SC], dt.bfloat16, kind="ExternalInput").ap()
    a_wc = nc.dram_tensor("wc", [128, WC_COLS], dt.bfloat16, kind="ExternalInput").ap()
    a_wf = nc.dram_tensor("wf", [4, WF_COLS], dt.float32, kind="ExternalInput").ap()
    a_bc = nc.dram_tensor("bc", [128, BC_COLS], dt.float32, kind="ExternalInput").ap()
    a_out = nc.dram_tensor("out", [128, NT * SC], dt.float32, kind="ExternalOutput").ap()

    with tile.TileContext(nc) as tc:
        _body(tc, a_m0, a_m1, a_m2, a_cst, a_icn, a_wc, a_wf, a_bc, a_out,
              w0, w1, w2)
    nc.compile()
    return nc


def _body(tc, a_m0, a_m1, a_m2, a_cst, a_icn, a_wc, a_wf, a_bc, a_out,
          w0, w1, w2):
    nc = tc.nc

    with ExitStack() as ctx:
        cpool = ctx.enter_context(tc.tile_pool(name="const", bufs=1))
        pin0 = ctx.enter_context(tc.tile_pool(name="in0", bufs=2))
        pin1 = ctx.enter_context(tc.tile_pool(name="in1", bufs=2))
        pin2 = ctx.enter_context(tc.tile_pool(name="in2", bufs=2))
        pinc = ctx.enter_context(tc.tile_pool(name="inc", bufs=2))
        ptnh = ctx.enter_context(tc.tile_pool(name="tnh", bufs=1))
        psml = ctx.enter_context(tc.tile_pool(name="sml", bufs=1))
        pout = ctx.enter_context(tc.tile_pool(name="out", bufs=2))
        # PSUM: chains 4 banks (bufs=1) + msgs 2 banks + experts 2 banks = 8
        pps_ch = ctx.enter_context(tc.tile_pool(name="pch", bufs=1, space="PSUM"))
        pps_m = ctx.enter_context(tc.tile_pool(name="pm", bufs=2, space="PSUM"))
        pps_e = ctx.enter_context(tc.tile_pool(name="pe", bufs=2, space="PSUM"))

        wc = cpool.tile([128, WC_COLS], dt.bfloat16, tag="wc")
        nc.sync.dma_start(wc[:], a_wc)
        wf = cpool.tile([4, WF_COLS], dt.float32, tag="wf")
        nc.sync.dma_start(wf[:], a_wf)
        bc = cpool.tile([128, BC_COLS], dt.float32, tag="bc")
        nc.sync.dma_start(bc[:], a_bc)

        def W(name):
            return wc[:, _wslot(name): _wslot(name) + 128]

        kron_g2 = wc[:, EX_G2:EX_G2 + 12]
        ones_sum = wc[0:12, EX_ONES:EX_ONES + 4]
        rb_f32 = wf[0:4, 0:12]
        scat = [wc[0:12, EX_SCAT + 128 * e: EX_SCAT + 128 * (e + 1)]
                for e in range(3)]

        b_loc4 = bc[:, 1:2]
        b_updh = bc[:, 2:3]   # 0.5 * b_upd, for sigmoid-via-tanh
        b_cnf4 = bc[:, 3:4]
        b_g14 = bc[:, 4:5]
        b_g2r = bc[0:12, 5:6]  # b_g2 on (g,e) rows 0..11
        b_msg4 = bc[:, 0:1]

        for i in range(NT):
            m0 = pin0.tile([128, w0 * SC], STAGE_DT, tag="m0")
            nc.sync.dma_start(m0[:], a_m0[:, i * w0 * SC:(i + 1) * w0 * SC])
            m1 = pin1.tile([128, w1 * SC], STAGE_DT, tag="m1")
            nc.sync.dma_start(m1[:], a_m1[:, i * w1 * SC:(i + 1) * w1 * SC])
            m2 = pin2.tile([128, w2 * SC], STAGE_DT, tag="m2")
            nc.sync.dma_start(m2[:], a_m2[:, i * w2 * SC:(i + 1) * w2 * SC])
            cst = pinc.tile([128, SC], dt.bfloat16, tag="cst")
            nc.sync.dma_start(cst[:], a_cst[:, i * SC:(i + 1) * SC])
            icn = pinc.tile([128, 3 * SC], dt.bfloat16, tag="icn")
            nc.sync.dma_start(icn[:], a_icn[:, i * 3 * SC:(i + 1) * 3 * SC])
            inv0 = icn[:, 0:SC]
            inv1 = icn[:, SC:2 * SC]
            inv2 = icn[:, 2 * SC:3 * SC]

            # ---- chain psums: St0 | St1 | St2 | agg (4 banks) ----
            pch = pps_ch.tile([128, 4 * SC], dt.float32, tag="ch")
            pSt0 = pch[:, 0:SC]
            pSt1 = pch[:, SC:2 * SC]
            pSt2 = pch[:, 2 * SC:3 * SC]
            pAgg = pch[:, 3 * SC:4 * SC]

            # raw tier-sum accumulation chains (identity stationary)
            for j in range(w0):
                nc.tensor.matmul(pSt0, W("I128"), m0[:, j * SC:(j + 1) * SC],
                                 start=(j == 0), stop=(j == w0 - 1))
            for j in range(w1):
                nc.tensor.matmul(pSt1, W("I128"), m1[:, j * SC:(j + 1) * SC],
                                 start=(j == 0), stop=(j == w1 - 1))
            for j in range(w2):
                nc.tensor.matmul(pSt2, W("I128"), m2[:, j * SC:(j + 1) * SC],
                                 start=(j == 0), stop=(j == w2 - 1))

            # ---- msgs: per-j matmul + tanh into SBUF, then accum chain ----
            tnh = ptnh.tile([128, w1 * SC], dt.bfloat16, tag="tnh")
            for j in range(w1):
                pm = pps_m.tile([128, SC], dt.float32, tag=f"pm")
                nc.tensor.matmul(pm[:], W("W4msg"), m1[:, j * SC:(j + 1) * SC],
                                 start=True, stop=True)
                nc.scalar.activation(tnh[:, j * SC:(j + 1) * SC], pm[:],
                                     AF.Tanh, bias=b_msg4, scale=1.0)
            for j in range(w1):
                nc.tensor.matmul(pAgg, W("I128"), tnh[:, j * SC:(j + 1) * SC],
                                 start=(j == 0), stop=(j == w1 - 1))

            # ---- means / S0 (blockT, bf16 operands for expert matmuls) ----
            mloc = psml.tile([128, SC], dt.bfloat16, tag="mloc")
            nc.vector.tensor_tensor(out=mloc[:], in0=pSt0, in1=inv0, op=ALU.mult)
            mdis = psml.tile([128, SC], dt.bfloat16, tag="mdis")
            nc.vector.tensor_tensor(out=mdis[:], in0=pSt2, in1=inv2, op=ALU.mult)
            aggb = psml.tile([128, SC], dt.bfloat16, tag="aggb")
            nc.vector.tensor_tensor(out=aggb[:], in0=pAgg, in1=inv1, op=ALU.mult)
            st1c = psml.tile([128, SC], dt.bfloat16, tag="st1c")
            nc.scalar.copy(st1c[:], pSt1)
            s01 = psml.tile([128, SC], dt.bfloat16, tag="s01")
            nc.vector.tensor_tensor(out=s01[:], in0=pSt0, in1=st1c[:], op=ALU.add)
            s0 = psml.tile([128, SC], dt.bfloat16, tag="s0")
            nc.vector.tensor_tensor(out=s0[:], in0=pSt2, in1=s01[:], op=ALU.add)

            # ---- local expert: tanh([cs, loc_mean] @ W_local + b) ----
            pl = pps_e.tile([128, SC], dt.float32, tag="pe")
            nc.tensor.matmul(pl[:], W("Wl_t"), cst[:], start=True, stop=False)
            nc.tensor.matmul(pl[:], W("Wl_b"), mloc[:], start=False, stop=True)
            locb = psml.tile([128, SC], dt.bfloat16, tag="locb")
            nc.scalar.activation(locb[:], pl[:], AF.Tanh, bias=b_loc4, scale=1.0)

            # ---- func expert: z = sigmoid(u) = 0.5*tanh(0.5u + 0.5b) + 0.5
            pu = pps_e.tile([128, SC], dt.float32, tag="pe")
            nc.tensor.matmul(pu[:], W("Wu_t"), cst[:], start=True, stop=False)
            nc.tensor.matmul(pu[:], W("Wu_b"), aggb[:], start=False, stop=True)
            tu = psml.tile([128, SC], dt.bfloat16, tag="tu")
            nc.scalar.activation(tu[:], pu[:], AF.Tanh, bias=b_updh, scale=0.5)
            tagg = psml.tile([128, SC], dt.bfloat16, tag="tagg")
            nc.scalar.activation(tagg[:], aggb[:], AF.Tanh)
            d2 = psml.tile([128, SC], dt.bfloat16, tag="d2")
            nc.vector.tensor_tensor(out=d2[:], in0=tagg[:], in1=cst[:], op=ALU.subtract)
            e1 = psml.tile([128, SC], dt.bfloat16, tag="e1")
            nc.vector.scalar_tensor_tensor(out=e1[:], in0=tu[:], scalar=0.5,
                                           in1=d2[:], op0=ALU.mult, op1=ALU.mult)
            e2 = psml.tile([128, SC], dt.bfloat16, tag="e2")
            nc.vector.scalar_tensor_tensor(out=e2[:], in0=d2[:], scalar=0.5,
                                           in1=cst[:], op0=ALU.mult, op1=ALU.add)
            funcb = psml.tile([128, SC], dt.bfloat16, tag="funcb")
            nc.vector.tensor_tensor(out=funcb[:], in0=e1[:], in1=e2[:], op=ALU.add)

            # ---- distant expert: 3 Euler steps, x kept bf16 ----
            xb = cst
            for s in range(N_STEPS):
                pc = pps_e.tile([128, SC], dt.float32, tag="pe")
                nc.tensor.matmul(pc[:], W("Wc_t"), xb[:], start=True, stop=False)
                nc.tensor.matmul(pc[:], W("Wc_b"), mdis[:], start=False, stop=True)
                vb = psml.tile([128, SC], dt.bfloat16, tag=f"vb{s}")
                nc.scalar.activation(vb[:], pc[:], AF.Tanh, bias=b_cnf4, scale=1.0)
                xn = psml.tile([128, SC], dt.bfloat16, tag=f"xn{s}")
                nc.vector.scalar_tensor_tensor(out=xn[:], in0=vb[:], scalar=DT_STEP,
                                               in1=xb[:], op0=ALU.mult, op1=ALU.add)
                xb = xn

            # ---- gating ----
            pg = pps_e.tile([128, SC], dt.float32, tag="pe")
            nc.tensor.matmul(pg[:], W("Wg1_t"), cst[:], start=True, stop=False)
            nc.tensor.matmul(pg[:], W("Wg1_b"), s0[:], start=False, stop=True)
            hb = psml.tile([128, SC], dt.bfloat16, tag="hb")
            nc.vector.tensor_scalar(out=hb[:], in0=pg[:], scalar1=b_g14,
                                    scalar2=0.0, op0=ALU.add, op1=ALU.max)
            pl2 = pps_e.tile([128, SC], dt.float32, tag="pe")
            nc.tensor.matmul(pl2[0:12, :], kron_g2, hb[:], start=True, stop=True)
            eg = psml.tile([12, SC], dt.bfloat16, tag="eg")
            nc.scalar.activation(eg[:], pl2[0:12, :], AF.Exp, bias=b_g2r, scale=1.0)
            ps = pps_e.tile([128, SC], dt.float32, tag="pe")
            nc.tensor.matmul(ps[0:4, :], ones_sum, eg[:], start=True, stop=True)
            rec = psml.tile([4, SC], dt.float32, tag="rec")
            nc.vector.reciprocal_approx_fast(out=rec[:], in_=ps[0:4, :])
            prb = pps_e.tile([128, SC], dt.float32, tag="pe")
            nc.tensor.matmul(prb[0:12, :], rb_f32, rec[:], start=True, stop=True)
            gts = psml.tile([12, SC], dt.bfloat16, tag="gts")
            nc.vector.tensor_tensor(out=gts[:], in0=eg[:], in1=prb[0:12, :], op=ALU.mult)

            # gate broadcast (12 -> 128 partitions) + weighted combine.
            # ge psums come one at a time from the msgs pool so the chain
            # banks free up right after the means (next ST's chains start).
            exps = [locb, funcb, xb]
            accs = []
            for e in range(3):
                pge = pps_m.tile([128, SC], dt.float32, tag="pm")
                nc.tensor.matmul(pge[:], scat[e], gts[:], start=True, stop=True)
                ae = psml.tile([128, SC], dt.bfloat16, tag=f"ae{e}")
                nc.vector.tensor_tensor(out=ae[:], in0=pge[:], in1=exps[e][:], op=ALU.mult)
                accs.append(ae)
            a1, a2, a3 = accs
            a12 = psml.tile([128, SC], dt.bfloat16, tag="a12")
            nc.vector.tensor_tensor(out=a12[:], in0=a1[:], in1=a2[:], op=ALU.add)
            outb = pout.tile([128, SC], dt.float32, tag="outb")
            nc.vector.tensor_tensor(out=outb[:], in0=a12[:], in1=a3[:], op=ALU.add)

            nc.sync.dma_start(a_out[:, i * SC:(i + 1) * SC], outb[:])


# ---------------------------------------------------------------------------
# host staging
# ---------------------------------------------------------------------------

def _to_blockT(arr_bsd):
    """[bs, d] (d == 32) -> blockT [128, NT*SC]: partition = g*32+d,
    cols = (i, t, c)."""
    bs, d = arr_bsd.shape
    a = arr_bsd.reshape(NT, TPS, 4, 32, d)           # [i, t, g, c, d]
    a = a.transpose(2, 4, 0, 1, 3)                   # [g, d, i, t, c]
    return np.ascontiguousarray(a.reshape(128, NT * SC))


def _nb_blockT(nb_sel):
    """[bs, w, 32] premasked sorted neighbors -> [128, NT*w*SC]:
    partition = g*32+d, cols = (i, j, t, c)."""
    bs, w, d = nb_sel.shape
    a = nb_sel.reshape(NT, TPS, 4, 32, w, d)         # [i, t, g, c, j, d]
    a = a.transpose(2, 5, 0, 4, 1, 3)                # [g, d, i, j, t, c]
    return np.ascontiguousarray(a.reshape(128, NT * w * SC))


def _from_blockT(arr):
    """inverse of _to_blockT: [128, NT*SC] -> [bs, 32]."""
    a = arr.reshape(4, 32, NT, TPS, 32)              # [g, d, i, t, c]
    a = a.transpose(2, 3, 0, 4, 1)                   # [i, t, g, c, d]
    return np.ascontiguousarray(a.reshape(NT * ST, 32))


def stage_weights(inputs, widths):
    f32 = np.float32
    W_local = np.asarray(inputs["W_local"], f32)
    W_msg = np.asarray(inputs["W_msg"], f32)
    W_upd = np.asarray(inputs["W_upd"], f32)
    W_cnf = np.asarray(inputs["W_cnf"], f32)
    W_g1 = np.asarray(inputs["W_g1"], f32)
    W_g2 = np.asarray(inputs["W_g2"], f32)

    eye4 = np.eye(4, dtype=f32)

    def kron4(w):
        return np.kron(eye4, w)

    wparts = {
        "I128": np.eye(128, dtype=f32),
        "W4msg": kron4(W_msg),
        "Wl_t": kron4(W_local[:D]), "Wl_b": kron4(W_local[D:]),
        "Wu_t": kron4(W_upd[:D]), "Wu_b": kron4(W_upd[D:]),
        "Wc_t": kron4(W_cnf[:D]), "Wc_b": kron4(W_cnf[D:]),
        "Wg1_t": kron4(W_g1[:D]), "Wg1_b": kron4(W_g1[D:] / K),
    }
    wc = np.zeros((128, WC_COLS), f32)
    for name in _WSLOTS:
        wc[:, _wslot(name):_wslot(name) + 128] = wparts[name]
    # kron(I4, W_g2): [128, 12]
    for g in range(4):
        wc[32 * g:32 * (g + 1), EX_G2 + 3 * g:EX_G2 + 3 * (g + 1)] = W_g2
    # ones_sum [12, 4]: row (g,e) -> col g
    for g in range(4):
        for e in range(3):
            wc[3 * g + e, EX_ONES + g] = 1.0
    # gate scatter: e fixed: [12, 128]: row (g,e') -> cols (g, d) if e'==e
    for e in range(3):
        for g in range(4):
            wc[3 * g + e, EX_SCAT + 128 * e + 32 * g:
               EX_SCAT + 128 * e + 32 * (g + 1)] = 1.0
    wc = wc.astype(bf16)

    wf = np.zeros((4, WF_COLS), f32)
    for g in range(4):
        wf[g, 3 * g:3 * (g + 1)] = 1.0   # recip bcast [4, 12]

    bcq = np.zeros((128, BC_COLS), f32)
    bcq[:, 0] = np.tile(np.asarray(inputs["b_msg"], f32), 4)
    bcq[:, 1] = np.tile(np.asarray(inputs["b_local"], f32), 4)
    bcq[:, 2] = 0.5 * np.tile(np.asarray(inputs["b_upd"], f32), 4)
    bcq[:, 3] = np.tile(np.asarray(inputs["b_cnf"], f32), 4)
    bcq[:, 4] = np.tile(np.asarray(inputs["b_g1"], f32), 4)
    b_g2 = np.asarray(inputs["b_g2"], f32)
    for g in range(4):
        bcq[3 * g:3 * (g + 1), 5] = b_g2
    return wc, wf, bcq


def stage_inputs(inputs):
    """Returns (in_maps, widths)."""
    f32 = np.float32
    cs = np.asarray(inputs["current_state"], f32)
    nb = np.asarray(inputs["neighbor_states"], f32)
    tiers = np.asarray(inputs["tier_ids"], np.int32)

    if np.any(np.asarray(inputs["b_msg"], f32) != 0.0):
        raise NotImplementedError("premask trick requires b_msg == 0")

    cnt = np.stack([(tiers == t).sum(-1) for t in range(3)], axis=1).astype(f32)  # [B, 3]
    widths = tuple(int(cnt[:, t].max()) for t in range(3))

    # per-tier sorted+premasked neighbor copies, truncated to widths
    copies = []
    for t in range(3):
        order = np.argsort(tiers != t, axis=1, kind="stable")[:, :widths[t]]
        sel = np.take_along_axis(nb, order[:, :, None], axis=1)
        msk = np.take_along_axis(tiers == t, order, axis=1)
        copies.append((sel * msk[:, :, None]).astype(STAGE_NP))

    inv = 1.0 / np.maximum(cnt, 1.0)       # [B, 3]

    wc, wf, bcq = stage_weights(inputs, widths)

    in_maps = []
    for c in range(N_CORES):
        rs = slice(c * BS, (c + 1) * BS)
        icn = np.empty((128, NT * 3 * SC), bf16)
        iv = [_to_blockT(np.repeat(inv[rs, t:t + 1], D, axis=1)) for t in range(3)]
        for i in range(NT):
            for t in range(3):
                icn[:, (3 * i + t) * SC:(3 * i + t + 1) * SC] = \
                    iv[t][:, i * SC:(i + 1) * SC]
        in_maps.append({
            "m0": _nb_blockT(copies[0][rs]),
            "m1": _nb_blockT(copies[1][rs]),
            "m2": _nb_blockT(copies[2][rs]),
            "cst": _to_blockT(cs[rs]).astype(bf16),
            "icn": icn,
            "wc": wc, "wf": wf, "bc": bcq,
        })
    return in_maps, widths


_PROGRAM_CACHE = {}


def kernel(**inputs):
    from concourse.bass_utils import run_bass_kernel_spmd

    in_maps, widths = stage_inputs(inputs)
    if widths not in _PROGRAM_CACHE:
        _PROGRAM_CACHE[widths] = build_program(*widths)
    nc = _PROGRAM_CACHE[widths]

    res = run_bass_kernel_spmd(nc, in_maps, core_ids=list(range(N_CORES)))
    out = np.concatenate([_from_blockT(np.asarray(r["out"], np.float32))
                          for r in res.results], axis=0)
    return out.astype(np.float32)
